# revision 8
# baseline (speedup 1.0000x reference)
"""AvgPool2d-as-Toeplitz kernel for Trainium2 (8 NeuronCores, SPMD).

The reference computes   out = (enc_x @ P.T) @ T.T   where P is the
zero-padding scatter matrix and T the Toeplitz matrix of a 3x3/stride-1
average pool over [C=8, H=32, W=32] images (entries 1/9, count_include_pad).
Both matrices are deterministic constants of the problem config, so the
kernel computes the pooling directly:

  out[b,c,h',w'] = (1/9) * sum_{dh,dw in {-1,0,1}} x_pad[b,c,h'+dh,w'+dw]

Sharding: data-parallel over batch B=64 -> 8 rows per core. Each core holds
64 images (8 batch x 8 channels) laid out in SBUF as
  [128 partitions = 4 images x 32 rows,  544 free = 16 groups x 34 (W+2 pad)]
The W-direction 3-tap sum is two vector-engine shifted adds along the free
dim (the zero pad columns make block boundaries correct). The H-direction
sum is one 128x128 block-diagonal banded matmul (4 x 32x32 tridiagonal
band, scaled by 1/9) on the tensor engine, contracting the partition dim.
"""

import numpy as np

B, C, H, W = 64, 8, 32, 32
N_CORES = 8
B_LOC = B // N_CORES          # batch rows per core
IMGS = B_LOC * C              # 64 images per core
SUB = 4                       # images stacked along the partition dim
GROUPS = IMGS // SUB          # 16 image groups along the free dim
WPAD = W + 2                  # 34
FREE = GROUPS * WPAD          # 544
PARTS = SUB * H               # 128
OUT_FREE = GROUPS * W         # 512

_CACHE = {}


def _avm() -> np.ndarray:
    # Block-diagonal [128,128]: 4 copies of the 32x32 tridiagonal band
    # (1 where |i-j|<=1), scaled by 1/9. Symmetric, so it is its own lhsT.
    idx = np.arange(H)
    band = (np.abs(idx[:, None] - idx[None, :]) <= 1).astype(np.float32)
    return np.kron(np.eye(SUB, dtype=np.float32), band) * np.float32(1.0 / 9.0)


IN_FREE = FREE + PARTS        # 672: [x layout | band matrix] fused in one buffer


def _build_nc():
    import concourse.tile as tile
    from concourse import bacc, mybir

    f32 = mybir.dt.float32
    # Bacc (not raw Bass): its compile() runs generate_event_semaphores(),
    # which splits multi-wait sync_info into ISA-legal chains (HW allows at
    # most 1 wait per instruction, 2 on InstEventSemaphore).
    nc = bacc.Bacc()
    # Single fused input (one DMA -> one DMA-sem lane): cols [0,544) are the
    # padded image layout, cols [544,672) the block-diagonal band matrix.
    # Keeping the kernel to 2 engines + 2 DMA lanes keeps the kernel-tail
    # drain within the ISA's sync-wait slot budget.
    x = nc.declare_dram_parameter("x", [PARTS, IN_FREE], f32, isOutput=False)
    y = nc.declare_dram_parameter("y", [PARTS, OUT_FREE], f32, isOutput=True)

    with tile.TileContext(nc) as tc:
        with (
            tc.tile_pool(name="sb", bufs=1) as pool,
            tc.tile_pool(name="ps", bufs=1, space="PSUM") as pp,
        ):
            xt = pool.tile([PARTS, IN_FREE], f32)
            nc.sync.dma_start(xt[:], x[:])

            # The fp32 matmul lowers to a fused LDWEIGHTS+MATMULT with a
            # single sync-wait slot; bounce the weights through the vector
            # engine so every matmul input comes from the DVE semaphore.
            wt = pool.tile([PARTS, PARTS], f32)
            nc.vector.tensor_copy(wt[:], xt[:, FREE:IN_FREE])

            # W-direction 3-tap sum: t2[:, j] = xt[:, j-1] + xt[:, j] + xt[:, j+1]
            # for j in [1, FREE-2]; pad columns (j % 34 in {0, 33}) are zero so
            # sums never leak across image groups.
            t1 = pool.tile([PARTS, FREE], f32)
            nc.vector.tensor_add(
                t1[:, 1 : FREE - 1], xt[:, 0 : FREE - 2], xt[:, 2:FREE]
            )
            t2 = pool.tile([PARTS, FREE], f32)
            nc.vector.tensor_add(
                t2[:, 1 : FREE - 1], t1[:, 1 : FREE - 1], xt[:, 1 : FREE - 1]
            )

            # H-direction banded sum (x 1/9): contract the partition dim with
            # the block-diagonal band. rhs reads only the 32 valid W columns
            # of each 34-wide group (strided AP), so N = 512 fits one matmul.
            acc = pp.tile([PARTS, OUT_FREE], f32)
            rhs = t2[:].rearrange("p (g w) -> p g w", w=WPAD)[:, :, 1 : 1 + W]
            nc.tensor.matmul(acc[:], wt[:], rhs, start=True, stop=True)

            ot = pool.tile([PARTS, OUT_FREE], f32)
            nc.vector.tensor_copy(ot[:], acc[:])
            nc.sync.dma_start(y[:], ot[:])

    nc.compile()
    return nc


def _get_nc():
    if "nc" not in _CACHE:
        _CACHE["nc"] = _build_nc()
    return _CACHE["nc"]


def _layout_core(xc: np.ndarray, avm: np.ndarray) -> np.ndarray:
    """[B_LOC, C*H*W] -> fused SBUF input [128, 672]: padded images | band."""
    g = xc.reshape(IMGS, H, W).reshape(GROUPS, SUB, H, W)
    gp = np.pad(g, ((0, 0), (0, 0), (0, 0), (1, 1)))
    X = gp.transpose(1, 2, 0, 3).reshape(PARTS, FREE)
    return np.ascontiguousarray(
        np.concatenate([X, avm], axis=1), dtype=np.float32
    )


def _unlayout_core(y: np.ndarray) -> np.ndarray:
    """[128, 512] SBUF layout -> [B_LOC, C*H*W]."""
    g = y.reshape(SUB, H, GROUPS, W).transpose(2, 0, 1, 3)
    return g.reshape(IMGS, H * W).reshape(B_LOC, C * H * W)


def kernel(enc_x: np.ndarray, weight: np.ndarray = None,
           padding_transform: np.ndarray = None, **_) -> np.ndarray:
    from concourse.bass_utils import run_bass_kernel_spmd

    enc_x = np.asarray(enc_x, dtype=np.float32)
    avm = _avm()
    in_maps = [
        {"x": _layout_core(enc_x[k * B_LOC : (k + 1) * B_LOC], avm)}
        for k in range(N_CORES)
    ]
    res = run_bass_kernel_spmd(_get_nc(), in_maps, list(range(N_CORES)))
    out = np.concatenate(
        [_unlayout_core(res.results[k]["y"]) for k in range(N_CORES)], axis=0
    )
    return out.astype(np.float32)


# revision 10
# speedup vs baseline: 1.0196x; 1.0196x over previous
"""AvgPool2d-as-Toeplitz kernel for Trainium2 (8 NeuronCores, SPMD).

The reference computes   out = (enc_x @ P.T) @ T.T   where P is the
zero-padding scatter matrix and T the Toeplitz matrix of a 3x3/stride-1
average pool over [C=8, H=32, W=32] images (entries 1/9, count_include_pad).
Both matrices are deterministic constants of the problem config, so the
kernel computes the pooling directly:

  out[b,c,h',w'] = (1/9) * sum_{dh,dw in {-1,0,1}} x_pad[b,c,h'+dh,w'+dw]

Sharding: data-parallel over batch B=64 -> 8 rows per core. Each core holds
64 images (8 batch x 8 channels) laid out in SBUF as
  [128 partitions = 4 images x 32 rows,  544 free = 16 groups x 34 (W+2 pad)]
The W-direction 3-tap sum is two vector-engine shifted adds along the free
dim (the zero pad columns make block boundaries correct). The H-direction
sum is one 128x128 block-diagonal banded matmul (4 x 32x32 tridiagonal
band, scaled by 1/9) on the tensor engine, contracting the partition dim.
"""

import numpy as np

B, C, H, W = 64, 8, 32, 32
N_CORES = 8
B_LOC = B // N_CORES          # batch rows per core
IMGS = B_LOC * C              # 64 images per core
SUB = 4                       # images stacked along the partition dim
GROUPS = IMGS // SUB          # 16 image groups along the free dim
WPAD = W + 2                  # 34
FREE = GROUPS * WPAD          # 544
PARTS = SUB * H               # 128
OUT_FREE = GROUPS * W         # 512

_CACHE = {}


def _avm() -> np.ndarray:
    # Block-diagonal [128,128]: 4 copies of the 32x32 tridiagonal band
    # (1 where |i-j|<=1), scaled by 1/9. Symmetric, so it is its own lhsT.
    idx = np.arange(H)
    band = (np.abs(idx[:, None] - idx[None, :]) <= 1).astype(np.float32)
    return np.kron(np.eye(SUB, dtype=np.float32), band) * np.float32(1.0 / 9.0)


IN_FREE = FREE + PARTS        # 672: [x layout | band matrix] fused in one buffer


def _build_nc():
    from concourse import bacc, mybir

    f32 = mybir.dt.float32
    nc = bacc.Bacc()
    # Single fused input (one DMA): cols [0,544) are the padded image
    # layout, cols [544,672) the block-diagonal band matrix.
    x = nc.declare_dram_parameter("x", [PARTS, IN_FREE], f32, isOutput=False)
    y = nc.declare_dram_parameter("y", [PARTS, OUT_FREE], f32, isOutput=True)

    with (
        nc.sbuf_tensor([PARTS, IN_FREE], f32) as xt,
        nc.sbuf_tensor([PARTS, FREE], f32) as t1,
        nc.sbuf_tensor([PARTS, FREE], f32) as t2,
        nc.sbuf_tensor([PARTS, OUT_FREE], f32) as ot,
        nc.psum_tensor([PARTS, OUT_FREE], f32) as acc,
        nc.semaphore() as s_in,
        nc.semaphore() as s_dve,
        nc.semaphore() as s_pe,
        nc.semaphore() as s_out,
        nc.Block() as block,
    ):

        @block.sync
        def _(sync):
            sync.dma_start(xt[:], x[:]).then_inc(s_in, 16)
            sync.wait_ge(s_dve, 3)
            sync.dma_start(y[:], ot[:]).then_inc(s_out, 16)
            sync.wait_ge(s_out, 16)

        @block.vector
        def _(vector):
            # W-direction 3-tap sum: t2[:, j] = xt[:,j-1] + xt[:,j] + xt[:,j+1]
            # for j in [1, FREE-2]; zero pad columns (j % 34 in {0, 33}) keep
            # sums from leaking across image groups.
            vector.wait_ge(s_in, 16)
            nc.vector.tensor_add(
                t1[:, 1 : FREE - 1], xt[:, 0 : FREE - 2], xt[:, 2:FREE]
            ).then_inc(s_dve)
            vector.wait_ge(s_dve, 1)
            nc.vector.tensor_add(
                t2[:, 1 : FREE - 1], t1[:, 1 : FREE - 1], xt[:, 1 : FREE - 1]
            ).then_inc(s_dve)
            vector.wait_ge(s_pe, 1)
            nc.vector.tensor_copy(ot[:], acc[:]).then_inc(s_dve)

        @block.tensor
        def _(tensor):
            # H-direction banded sum (x 1/9): contract the partition dim with
            # the block-diagonal band (lhsT is a view into the fused input).
            # rhs reads only the 32 valid W columns of each 34-wide group
            # (strided AP), so N = 512 fits one fp32 matmul.
            tensor.wait_ge(s_dve, 2)
            rhs = t2[:].rearrange("p (g w) -> p g w", w=WPAD)[:, :, 1 : 1 + W]
            nc.tensor.matmul(
                acc[:], xt[:, FREE:IN_FREE], rhs, start=True, stop=True
            ).then_inc(s_pe)

    nc.compile()
    return nc


def _get_nc():
    if "nc" not in _CACHE:
        _CACHE["nc"] = _build_nc()
    return _CACHE["nc"]


def _layout_core(xc: np.ndarray, avm: np.ndarray) -> np.ndarray:
    """[B_LOC, C*H*W] -> fused SBUF input [128, 672]: padded images | band."""
    g = xc.reshape(IMGS, H, W).reshape(GROUPS, SUB, H, W)
    gp = np.pad(g, ((0, 0), (0, 0), (0, 0), (1, 1)))
    X = gp.transpose(1, 2, 0, 3).reshape(PARTS, FREE)
    return np.ascontiguousarray(
        np.concatenate([X, avm], axis=1), dtype=np.float32
    )


def _unlayout_core(y: np.ndarray) -> np.ndarray:
    """[128, 512] SBUF layout -> [B_LOC, C*H*W]."""
    g = y.reshape(SUB, H, GROUPS, W).transpose(2, 0, 1, 3)
    return g.reshape(IMGS, H * W).reshape(B_LOC, C * H * W)


def kernel(enc_x: np.ndarray, weight: np.ndarray = None,
           padding_transform: np.ndarray = None, **_) -> np.ndarray:
    from concourse.bass_utils import run_bass_kernel_spmd

    enc_x = np.asarray(enc_x, dtype=np.float32)
    avm = _avm()
    in_maps = [
        {"x": _layout_core(enc_x[k * B_LOC : (k + 1) * B_LOC], avm)}
        for k in range(N_CORES)
    ]
    res = run_bass_kernel_spmd(_get_nc(), in_maps, list(range(N_CORES)))
    out = np.concatenate(
        [_unlayout_core(res.results[k]["y"]) for k in range(N_CORES)], axis=0
    )
    return out.astype(np.float32)
